# revision 17
# baseline (speedup 1.0000x reference)
"""Multi-head attention (B=2, S=2048, D=1024, H=16) on 8 trn2 NeuronCores.

Sharding: core c -> batch b = c//4, head-group g = c%4 (4 heads each).
Each core: QKV projections for its 256 output dims, causal attention for its
4 heads, partial output projection over its 256 contraction dims.
Host: sum the 4 partial outputs per batch, add (bo + bv @ wo.T).

v2 design (vs the 226us baseline):
- All device layouts prearranged on host so every DMA is a single large
  per-partition-contiguous transfer (>=0.25MB, 4KB+ rows): q/k/v chunk-major
  bf16, weights p-major bf16, y out fp16 chunk-major. (fp8 anywhere in the
  QK or V path was measured at 2-3e-2 partial error -- exp amplifies score
  noise -- so everything stays bf16.)
- Attention inner loop is software-pipelined: scores for entry j+1 are
  emitted BEFORE PV of entry j, with a double-buffered score PSUM, so the
  PE never sits behind the 1.1us exp on the scalar engine.
- ACT engine runs exp ONLY (l / rl_bc copies moved to DVE) since total exp
  time (~82us) is within ~7us of the PE floor (~88us).
- PSUM budget exactly 8 banks: s(2x2) + po(1x2) + shared w(1x2).
"""

import os
import hashlib
import numpy as np

B, S, D, H, DK = 2, 2048, 1024, 16, 64
NCORES = 8
GROUPS = 4          # head groups per batch
HPG = 4             # heads per group (per core)
GDIM = HPG * DK     # 256 output dims per core
NEG = -1.0e9
QB = 512            # q block width
NQB = S // QB       # 4
NKT = S // 128      # 16 k tiles
NDM = D // 128      # 8 contraction tiles for projections

MODE = os.environ.get("BASS_MHA_MODE", "bf16")

_CACHE = {}


def _make_plan(m2d):
    """Classify 128x128 blocks of the (q,k) mask into skip/full/mixed.

    Returns per (qb, j): (j, cmin_local, bias_cols) where bias_cols is a list
    of (c_local, uniq_tile_idx); plus the packed unique bias blocks.
    """
    sub = np.asarray(m2d).reshape(S // 128, 128, S // 128, 128)
    any_ = sub.any(axis=(1, 3))   # [qtile, ktile]
    all_ = sub.all(axis=(1, 3))

    uniq = {}
    uniq_src = []
    plan = []
    for qb in range(NQB):
        entries = []
        cs = list(range(4 * qb, 4 * qb + 4))
        for j in range(NKT):
            states = []
            for c in cs:
                if not any_[c, j]:
                    states.append("skip")
                elif all_[c, j]:
                    states.append("full")
                else:
                    states.append("mixed")
            if all(s == "skip" for s in states):
                continue
            cmin = next(i for i, s in enumerate(states) if s != "skip")
            bias_cols = []
            for i in range(cmin, 4):
                if states[i] == "full":
                    continue
                c = cs[i]
                if states[i] == "skip":
                    blk = np.full((128, 128), NEG, np.float32)
                else:
                    m = sub[c, :, j, :]  # [128 q, 128 k]
                    blk = np.where(m.T != 0, 0.0, NEG).astype(np.float32)
                hsh = hashlib.sha1(blk.tobytes()).hexdigest()
                if hsh not in uniq:
                    uniq[hsh] = len(uniq_src)
                    uniq_src.append(blk)
                bias_cols.append((i, uniq[hsh]))
            entries.append((j, cmin, bias_cols))
        plan.append(entries)
    bias_pack = (
        np.stack(uniq_src) if uniq_src else np.zeros((1, 128, 128), np.float32)
    )
    key = hashlib.sha1(
        repr([(qb, e) for qb, e in enumerate(plan)]).encode()
    ).hexdigest()
    return plan, bias_pack, key


def _build(mode, plan, n_bias):
    import concourse.mybir as mybir
    from concourse import bacc, tile

    f32 = mybir.dt.float32
    bf16 = mybir.dt.bfloat16
    fp16 = mybir.dt.float16
    qk_dt = bf16

    AF = mybir.ActivationFunctionType
    AO = mybir.AluOpType

    nc = bacc.Bacc(
        "TRN2", target_bir_lowering=False, debug=False, num_devices=NCORES
    )

    # host-prearranged layouts; [128, ...] = partition-major, contiguous rows
    qc_d = nc.declare_dram_parameter("qc", [128, NQB, 8 * QB], qk_dt, isOutput=False).ap()
    kc_d = nc.declare_dram_parameter("kc", [128, NQB, 8 * QB], qk_dt, isOutput=False).ap()
    vc_d = nc.declare_dram_parameter("vc", [128, NQB, 8 * QB], bf16, isOutput=False).ap()
    wq_d = nc.declare_dram_parameter("wq2", [128, NDM, GDIM], qk_dt, isOutput=False).ap()
    wk_d = nc.declare_dram_parameter("wk2", [128, NDM, GDIM], qk_dt, isOutput=False).ap()
    wv_d = nc.declare_dram_parameter("wv2", [128, NDM, GDIM], bf16, isOutput=False).ap()
    wo_d = nc.declare_dram_parameter("wo2", [128, 2, D], bf16, isOutput=False).ap()
    bq_d = nc.declare_dram_parameter("bq2", [128, 2], f32, isOutput=False).ap()
    bk_d = nc.declare_dram_parameter("bk2", [128, 2], f32, isOutput=False).ap()
    bias_d = nc.declare_dram_parameter(
        "bias_pack", [128, n_bias, 2, 128], f32, isOutput=False
    ).ap()
    y_d = nc.declare_dram_parameter("y", [NQB, 128, 8 * QB], fp16, isOutput=True).ap()

    with tile.TileContext(nc) as tc:
        with (
            tc.tile_pool(name="res", bufs=1) as res,
            tc.tile_pool(name="ot_pool", bufs=2) as ot_pool,
            tc.tile_pool(name="ptp", bufs=4) as ptp,
            tc.tile_pool(name="ystage", bufs=2) as ystage,
            tc.tile_pool(name="small", bufs=4) as small,
            tc.tile_pool(name="psum", bufs=1, space="PSUM") as psum,
        ):
            dma = nc.sync.dma_start
            odma = nc.sync.dma_start

            # ---- resident tiles ----
            wq_sb = res.tile([128, NDM, GDIM], qk_dt, name="wq_sb")
            wk_sb = res.tile([128, NDM, GDIM], qk_dt, name="wk_sb")
            wv_sb = res.tile([128, NDM, GDIM], bf16, name="wv_sb")
            wo_sb = res.tile([128, 2, D], bf16, name="wo_sb")
            bq_sb = res.tile([128, 2], f32, name="bq_sb")
            bk_sb = res.tile([128, 2], f32, name="bk_sb")
            bias_sb = res.tile([128, n_bias, 2, 128], f32, name="bias_sb")
            qc_sb = [res.tile([128, NDM, QB], qk_dt, name=f"qc{c}") for c in range(4)]
            kc_sb = [res.tile([128, NDM, QB], qk_dt, name=f"kc{c}") for c in range(4)]
            vc_sb = [res.tile([128, NDM, QB], bf16, name=f"vc{c}") for c in range(4)]

            QT_c = [res.tile([128, 2, QB], bf16, name=f"QT{i}") for i in range(4)]
            KT_c = [res.tile([128, 2, QB], bf16, name=f"KT{i}") for i in range(4)]
            V_c = [
                res.tile([128, 4, HPG, DK + 1], bf16, name=f"V{i}")
                for i in range(4)
            ]
            for i in range(4):
                nc.vector.memset(V_c[i][:, :, :, DK : DK + 1], 1.0)
            ones64_sb = res.tile([1, 64], f32, name="ones64_sb")
            nc.vector.memset(ones64_sb, 1.0)

            # ---- input DMAs, in the order compute needs them ----
            dma(out=wq_sb, in_=wq_d)
            dma(out=wk_sb, in_=wk_d)
            dma(out=bq_sb, in_=bq_d)
            dma(out=bk_sb, in_=bk_d)
            def in_chunk(c):
                dma(out=qc_sb[c], in_=qc_d[:, c, :].rearrange("p (dm q) -> p dm q", q=QB))
                dma(out=kc_sb[c], in_=kc_d[:, c, :].rearrange("p (dm q) -> p dm q", q=QB))
            in_chunk(0)
            in_chunk(1)
            dma(out=bias_sb, in_=bias_d)
            dma(out=wv_sb, in_=wv_d)
            dma(out=vc_sb[0], in_=vc_d[:, 0, :].rearrange("p (dm q) -> p dm q", q=QB))
            dma(out=vc_sb[1], in_=vc_d[:, 1, :].rearrange("p (dm q) -> p dm q", q=QB))
            dma(out=wo_sb, in_=wo_d)
            for c in (2, 3):
                in_chunk(c)
                dma(out=vc_sb[c], in_=vc_d[:, c, :].rearrange("p (dm q) -> p dm q", q=QB))

            # ---- filler framework: single-MM units of projection /
            # out-projection work, striped a few per attention entry so the
            # PE stream stays dense while ACT paces the exp chain ----
            from collections import deque
            fillers = deque()  # (emit_fn, category)

            def qk_units(ci, which, ot):
                x_sb, w_sb, b_sb, dst = (
                    (qc_sb[ci], wq_sb, bq_sb, QT_c[ci]) if which == 0
                    else (kc_sb[ci], wk_sb, bk_sb, KT_c[ci])
                )
                cell = {}
                def mk(dm):
                    def emit():
                        if dm == 0:
                            cell['ps'] = psum.tile(
                                [128, QB], f32, name="ps_p", tag="w", bufs=2
                            )
                        nc.tensor.matmul(
                            cell['ps'],
                            lhsT=w_sb[:, dm, 128 * ot : 128 * (ot + 1)],
                            rhs=x_sb[:, dm, :],
                            start=(dm == 0),
                            stop=(dm == NDM - 1),
                        )
                        if dm == NDM - 1:
                            nc.vector.tensor_scalar_add(
                                dst[:, ot, :], cell['ps'], b_sb[:, ot : ot + 1]
                            )
                    return emit
                return [mk(dm) for dm in range(NDM)]

            def v_units(ci, rt):
                cell = {}
                def mk(dm):
                    def emit():
                        if dm == 0:
                            cell['ps'] = psum.tile(
                                [128, GDIM], f32, name="ps_v", tag="w", bufs=2
                            )
                        nc.tensor.matmul(
                            cell['ps'],
                            lhsT=vc_sb[ci][:, dm, 128 * rt : 128 * (rt + 1)],
                            rhs=wv_sb[:, dm, :],
                            start=(dm == 0),
                            stop=(dm == NDM - 1),
                        )
                        if dm == NDM - 1:
                            nc.vector.tensor_copy(
                                out=V_c[ci][:, rt, :, 0:DK],
                                in_=cell['ps'].rearrange("p (h d) -> p h d", d=DK),
                            )
                    return emit
                return [mk(dm) for dm in range(NDM)]

            def proj_fillers(ci):
                cat = f"p{ci}"
                for which in (0, 1):
                    for ot in range(2):
                        for u in qk_units(ci, which, ot):
                            fillers.append((u, cat))
                for rt in range(4):
                    for u in v_units(ci, rt):
                        fillers.append((u, cat))

            def proj_qk(ci):
                for which in (0, 1):
                    for ot in range(2):
                        for u in qk_units(ci, which, ot):
                            u()

            def proj_v(ci):
                for rt in range(4):
                    for u in v_units(ci, rt):
                        u()

            def drain(n):
                for _ in range(min(n, len(fillers))):
                    fillers.popleft()[0]()

            def flush(cat):
                idx = None
                for i, (_, c) in enumerate(fillers):
                    if c == cat:
                        idx = i
                if idx is not None:
                    drain(idx + 1)

            def attn_pair(qb, pr, OT_sb):
                """Scores+softmax+PV for heads (2pr, 2pr+1) of q-block qb.

                Emission is software-pipelined: S(j+1) goes to the PE queue
                before PV(j), so the PE streams scores while ACT runs exp.
                """
                entries = plan[qb]
                last_j = entries[-1][0]
                first_j = entries[0][0]
                heads = (2 * pr, 2 * pr + 1)
                po = {}
                for h in heads:
                    po[h] = psum.tile(
                        [DK + 1, QB], f32, name=f"po{h}", tag="po", bufs=2
                    )
                pend = None  # (j, off, pt) awaiting PV emission

                def emit_pv(j, off, pt):
                    jc, jl = j // 4, j % 4
                    for hh, h in enumerate(heads):
                        nc.tensor.matmul(
                            po[h][:, off:QB],
                            lhsT=V_c[jc][:, jl, h, :],
                            rhs=pt[:, hh, off:QB],
                            start=(j == first_j),
                            stop=(j == last_j),
                        )

                for j, cmin, bias_cols in entries:
                    off = 128 * cmin
                    jc, jl = j // 4, j % 4
                    ps_s = psum.tile([128, 2, QB], f32, name="ps_s", tag="s", bufs=2)
                    for hh, h in enumerate(heads):
                        p0 = 64 * hh
                        ht = h // 2
                        nc.tensor.matmul(
                            ps_s[:, hh, off:QB],
                            lhsT=KT_c[jc][
                                p0 : p0 + 64, ht, 128 * jl : 128 * (jl + 1)
                            ],
                            rhs=QT_c[qb][p0 : p0 + 64, ht, off:QB],
                            start=True,
                            stop=True,
                        )
                    for cl, ui in bias_cols:
                        co = 128 * cl
                        nc.vector.tensor_tensor(
                            out=ps_s[:, :, co : co + 128],
                            in0=ps_s[:, :, co : co + 128],
                            in1=bias_sb[:, ui, :, :],
                            op=AO.add,
                        )
                    pt = ptp.tile([128, 2, QB], bf16, name="pt")
                    nc.scalar.activation(
                        pt[:, :, off:QB],
                        ps_s[:, :, off:QB],
                        AF.Exp,
                        scale=0.125,
                    )
                    # fillers go between S(j) and PV(j-1): they run on the PE
                    # during exp(j-1)'s latency, which PV(j-1) must wait out
                    drain(2 + (len(fillers) > 64))
                    if pend is not None:
                        emit_pv(*pend)
                    pend = (j, off, pt)
                emit_pv(*pend)

                # normalize: 1/l per head, replicate across 64 partitions
                ps_rl = psum.tile([128, QB], f32, name="ps_rl", tag="w", bufs=2)
                for hh, h in enumerate(heads):
                    # NB: custom DVE ops (reciprocal) read garbage from PSUM
                    # on HW -- copy the denominator row to SBUF first
                    l1 = small.tile([1, QB], f32, name="l1", tag=f"l{hh}")
                    nc.vector.tensor_copy(out=l1, in_=po[h][DK : DK + 1, :])
                    rl1 = small.tile([1, QB], f32, name="rl1", tag=f"rl{hh}")
                    nc.vector.reciprocal_approx_fast(out=rl1, in_=l1)
                    nc.tensor.matmul(
                        ps_rl[64 * hh : 64 * hh + 64, :],
                        lhsT=ones64_sb,
                        rhs=rl1,
                        start=True,
                        stop=True,
                    )
                rl_bc = small.tile([128, QB], f32, name="rl_bc", tag="rl_bc")
                nc.vector.tensor_copy(out=rl_bc, in_=ps_rl)
                for hh, h in enumerate(heads):
                    p0 = 64 * hh
                    nc.vector.tensor_tensor(
                        out=OT_sb[p0 : p0 + 64, h // 2, :],
                        in0=po[h][0:DK, :],
                        in1=rl_bc[p0 : p0 + 64, :],
                        op=AO.mult,
                    )

            def y_units(qb, OT_sb, ysb, ot8):
                cell = {}
                def mk(ct):
                    def emit():
                        if ct == 0:
                            cell['ps'] = psum.tile(
                                [128, QB], f32, name="ps_y", tag="w", bufs=2
                            )
                        nc.tensor.matmul(
                            cell['ps'],
                            lhsT=wo_sb[:, ct, 128 * ot8 : 128 * (ot8 + 1)],
                            rhs=OT_sb[:, ct, :],
                            start=(ct == 0),
                            stop=(ct == 1),
                        )
                        if ct == 1:
                            nc.vector.tensor_copy(
                                out=ysb[:, ot8, :], in_=cell['ps']
                            )
                            if ot8 == 7:
                                odma(
                                    out=y_d[qb],
                                    in_=ysb.rearrange("p dm q -> p (dm q)"),
                                )
                    return emit
                return [mk(0), mk(1)]

            def outproj_fillers(qb, OT_sb):
                ysb = ystage.tile([128, 8, QB], fp16, name="ysb")
                for ot8 in range(8):
                    for u in y_units(qb, OT_sb, ysb, ot8):
                        fillers.append((u, f"y{qb}"))

            OTm = {}
            for qb in range(4):
                OTm[qb] = ot_pool.tile(
                    [128, 2, QB], bf16, name=f"OT{qb}", tag="OT", bufs=2
                )
            proj_qk(0)
            proj_v(0)
            attn_pair(0, 0, OTm[0])
            proj_fillers(1)
            attn_pair(0, 1, OTm[0])
            flush("p1")  # chunk-1 proj must precede attn(1,*) in the PE queue
            proj_fillers(2)
            outproj_fillers(0, OTm[0])
            attn_pair(1, 0, OTm[1])
            proj_fillers(3)
            attn_pair(1, 1, OTm[1])
            flush("p2")
            outproj_fillers(1, OTm[1])
            attn_pair(2, 0, OTm[2])
            attn_pair(2, 1, OTm[2])
            flush("p3")
            outproj_fillers(2, OTm[2])
            attn_pair(3, 0, OTm[3])
            attn_pair(3, 1, OTm[3])
            drain(999)
            outproj_fillers(3, OTm[3])
            drain(999)

    nc.compile()
    return nc


def _get_nc(mode, plan, n_bias, key):
    ck = (mode, key, n_bias)
    if ck not in _CACHE:
        _CACHE[ck] = _build(mode, plan, n_bias)
    return _CACHE[ck]


def _chunk_major(xT, np_dt):
    """[1024, 2048] f32 -> [128, 4 ci, 8 dm * 512] in np_dt."""
    a = np.asarray(xT, np.float32).reshape(NDM, 128, NQB, QB)
    a = a.transpose(1, 2, 0, 3).reshape(128, NQB, NDM * QB)
    return np.ascontiguousarray(a).astype(np_dt)


def _prep_inputs(q, k, v, wq, bq, wk, bk, wv, wo, bias_pack, mode):
    """Build the 8 per-core input maps."""
    import ml_dtypes

    f32 = np.float32
    qk_np = ml_dtypes.bfloat16
    bf16 = ml_dtypes.bfloat16

    wqT = np.ascontiguousarray(np.asarray(wq, f32).T)
    wkT = np.ascontiguousarray(np.asarray(wk, f32).T)
    wvT = np.ascontiguousarray(np.asarray(wv, f32).T)
    woT = np.ascontiguousarray(np.asarray(wo, f32).T)
    bp = np.asarray(bias_pack, f32).transpose(1, 0, 2)  # [128, n, 128]
    bias_pm = np.ascontiguousarray(
        np.repeat(bp[:, :, None, :], 2, axis=2)
    )  # [128, n, 2, 128] -- duplicated so one DVE add covers both heads

    def pm_weight(wT_s):
        # [1024, 256] -> [128 p, 8 dm, 256 m]; d = p + 128*dm
        a = wT_s.reshape(NDM, 128, GDIM).transpose(1, 0, 2)
        return np.ascontiguousarray(a).astype(qk_np)

    in_maps = []
    per_batch = {}
    for b in range(B):
        per_batch[b] = (
            _chunk_major(np.asarray(q[b], f32).T, qk_np),
            _chunk_major(np.asarray(k[b], f32).T, qk_np),
            _chunk_major(np.asarray(v[b], f32).T, bf16),
        )
    for c in range(NCORES):
        b, g = c // GROUPS, c % GROUPS
        sl = slice(GDIM * g, GDIM * (g + 1))
        qcm, kcm, vcm = per_batch[b]
        im = {
            "qc": qcm,
            "kc": kcm,
            "vc": vcm,
            "wq2": pm_weight(wqT[:, sl]),
            "wk2": pm_weight(wkT[:, sl]),
            "wv2": np.ascontiguousarray(
                wvT[:, sl].reshape(NDM, 128, GDIM).transpose(1, 0, 2)
            ).astype(bf16),
            "wo2": np.ascontiguousarray(
                woT[sl, :].reshape(2, 128, D).transpose(1, 0, 2)
            ).astype(bf16),
            "bq2": np.ascontiguousarray(
                np.asarray(bq, f32)[sl].reshape(2, 128).T
            ),
            "bk2": np.ascontiguousarray(
                np.asarray(bk, f32)[sl].reshape(2, 128).T
            ),
            "bias_pack": bias_pm,
        }
        in_maps.append(im)
    return in_maps


def _decode_y(y_raw):
    """[4 qb, 128, 8 dm * 512] fp16 -> yT [1024, 2048] f32."""
    a = np.asarray(y_raw, np.float32).reshape(NQB, 128, NDM, QB)
    return a.transpose(2, 1, 0, 3).reshape(D, S)


def _kernel_impl(q, k, v, mask, wq, bq, wk, bk, wv, bv, wo, bo, trace=False):
    from concourse.bass_utils import run_bass_kernel_spmd

    f32 = np.float32
    m2d = np.asarray(mask)[0, 0]
    plan, bias_pack, key = _make_plan(m2d)
    nc = _get_nc(MODE, plan, bias_pack.shape[0], key)
    in_maps = _prep_inputs(q, k, v, wq, bq, wk, bk, wv, wo, bias_pack, MODE)

    res = run_bass_kernel_spmd(nc, in_maps, list(range(NCORES)), trace=trace)

    bo_eff = (
        np.asarray(bo, np.float64)
        + np.asarray(bv, np.float64) @ np.asarray(wo, np.float64).T
    ).astype(f32)

    out = np.zeros((B, S, D), f32)
    for c in range(NCORES):
        out[c // GROUPS] += _decode_y(res.results[c]["y"]).T
    out += bo_eff
    return out, res


def kernel(q, k, v, mask, wq, bq, wk, bk, wv, bv, wo, bo):
    out, _ = _kernel_impl(q, k, v, mask, wq, bq, wk, bk, wv, bv, wo, bo)
    return out
